# revision 38
# baseline (speedup 1.0000x reference)
"""Trainium2 Bass kernel for nn_NextRowPredictionHead (loss_fn).

Feature-parallel across 8 cores (4 cat + 2 num + 2 bool features each,
full batch). Per core the work is phased per feature so the ACT engine
loads each activation table set once per phase instead of thrashing:

  per f: A) shared matmul + Gelu            [gelu_and_others]
         B) LN stats + Square/Sqrt          [sqrt_and_others]
         C) heads + Exp/softmax-accum       [exp_and_others]
  end:   D) deferred Ln + loss assembly     [natural_log]

Other structure vs v0:
  - all inputs host-cast to bf16 (halves HBM traffic, kills DMA casts)
  - stat rows (mu/se/lam/qt) go row->column via K=1 rank-1 matmuls into
    PSUM instead of DRAM round-trip transposes
  - Relu and row copies moved from ACT to DVE
  - weights host-pre-rearranged so every DMA is contiguous HWDGE
"""

import sys
from contextlib import ExitStack

import numpy as np
from ml_dtypes import bfloat16

sys.path.insert(0, "/opt/trn_rl_repo")

import concourse.bass as bass  # noqa: E402
import concourse.tile as tile  # noqa: E402
from concourse import bacc, mybir  # noqa: E402
from concourse.bass_utils import run_bass_kernel_spmd  # noqa: E402

F32 = mybir.dt.float32
BF16 = mybir.dt.bfloat16
AF = mybir.ActivationFunctionType
OP = mybir.AluOpType

P = 128
D = 512
H = 256
V = 1000
B = 2048
NC, NN, NB = 32, 16, 16
FC, FN, FB = 4, 2, 2
NF = FC + FN + FB
NBT = 4
BT = 512
NBS = 4
NS = NBT * NBS
LN_EPS = 1e-5
N_CORES = 8

LAST_RESULTS = None


def _build():
    nc = bacc.Bacc("TRN2", target_bir_lowering=False, debug=False,
                   num_devices=N_CORES)
    io = {}

    def din(name, shape, dt=BF16):
        io[name] = nc.dram_tensor(name, shape, dt, kind="ExternalInput").ap()

    din("xt", [NF, D, B])
    din("w1t", [P, 4, 4, P])
    din("b1c", [P, 4], F32)
    din("w1f", [NF, P, 4, 2, P])
    din("w1r", [1, NF, 2, H])
    din("w2c", [FC, P, 2, V])
    din("wct", [FC, P, 2, B])
    din("w2n", [FN, P, 2, D])
    din("w2b", [FB, P, 2, 2])
    din("b2all", [1, FC * V + FN * D + FB * 2])
    din("tgt", [FN, B, D])
    din("mask_all", [P, NF, NS], F32)
    din("aux_all", [P, NF, NS], F32)
    out = nc.dram_tensor("loss_out", [P, 2 * NF], F32,
                     kind="ExternalOutput").ap()

    with tile.TileContext(nc) as tc:
        with ExitStack() as ctx:
            build_body(ctx, tc, io, out)
    nc.compile()
    return nc


def build_body(ctx, tc, io, out):
    nc = tc.nc

    const = ctx.enter_context(tc.tile_pool(name="const", bufs=1))
    wpool = ctx.enter_context(tc.tile_pool(name="wpool", bufs=3))
    xpool = ctx.enter_context(tc.tile_pool(name="xpool", bufs=2))
    hpool = ctx.enter_context(tc.tile_pool(name="hpool", bufs=3))
    rows = ctx.enter_context(tc.tile_pool(name="rows", bufs=3))
    work = ctx.enter_context(tc.tile_pool(name="work", bufs=2))
    tgp = ctx.enter_context(tc.tile_pool(name="tgp", bufs=3))
    cols = ctx.enter_context(tc.tile_pool(name="cols", bufs=3))
    fin = ctx.enter_context(tc.tile_pool(name="fin", bufs=1))
    dpool = ctx.enter_context(tc.tile_pool(name="dram", bufs=1, space="DRAM"))
    # ps_mm is shared by phase A (shared-layer) and phase C (head2): the
    # WAR rotation throttles next-group shared MMs until this group's
    # Exps drain, which keeps Gelu/Exp from interleaving on ACT (each
    # flip would cost a ~1.4us activation-table load).
    ps_mm = ctx.enter_context(tc.tile_pool(name="ps_mm", bufs=2, space="PSUM"))
    ps_st = ctx.enter_context(tc.tile_pool(name="ps_st", bufs=2, space="PSUM"))
    ps_c8 = ctx.enter_context(tc.tile_pool(name="ps_c8", bufs=1, space="PSUM"))
    ps_h1 = ctx.enter_context(tc.tile_pool(name="ps_h1", bufs=2, space="PSUM"))

    # ---- constants, loaded once ----
    ones_bf = const.tile([P, 1], BF16)
    nc.vector.memset(ones_bf, 1.0)
    eps_t = const.tile([1, 1], F32)
    nc.vector.memset(eps_t, LN_EPS)
    w1_t = const.tile([P, 4, 4, P], BF16)
    nc.sync.dma_start(out=w1_t, in_=io["w1t"])
    b1_t = const.tile([P, 4], F32)
    nc.scalar.dma_start(out=b1_t, in_=io["b1c"])
    w1r_t = const.tile([1, NF, 2, H], BF16)
    nc.scalar.dma_start(out=w1r_t, in_=io["w1r"])
    nb2 = FC * V + FN * D + FB * 2
    b2_t = const.tile([1, nb2], BF16)
    nc.scalar.dma_start(out=b2_t, in_=io["b2all"])
    mask_t = const.tile([P, NF, NS], F32)
    nc.scalar.dma_start(out=mask_t, in_=io["mask_all"])
    aux_t = const.tile([P, NF, NS], F32)
    nc.scalar.dma_start(out=aux_t, in_=io["aux_all"])
    ssum_all = const.tile([P, NF, NS], F32)
    t1_all = const.tile([P, NF, NS], F32)
    ceb_all = const.tile([P, NF, NS], F32)
    ceacc = const.tile([P, 2 * NF], F32)
    for f in range(NF):
        nc.vector.reduce_sum(ceacc[:, NF + f:NF + f + 1], mask_t[:, f, :],
                             axis=mybir.AxisListType.X)

    def feat_kind(f):
        if f < FC:
            return "c", f
        if f < FC + FN:
            return "n", f - FC
        return "b", f - FC - FN

    def b2row_of(f):
        kind, j = feat_kind(f)
        if kind == "c":
            return b2_t[0:1, j * V:(j + 1) * V]
        if kind == "n":
            return b2_t[0:1, FC * V + j * D:FC * V + (j + 1) * D]
        return b2_t[0:1, FC * V + FN * D + 2 * j:FC * V + FN * D + 2 * j + 2]

    hraw_t, w1f_t = {}, {}
    murow_t, serow_t, lamcol_t = {}, {}, {}

    def phase_a(f):
        xt_t = xpool.tile([P, 4, B], BF16, tag="xt")
        for bt in range(NBT):
            bsl = slice(bt * BT, (bt + 1) * BT)
            nc.sync.dma_start(
                out=xt_t[:, :, bsl],
                in_=io["xt"][f][:, bsl].rearrange("(dc p) b -> p dc b", p=P))
        w1_f = wpool.tile([P, 4, 2, P], BF16, tag="w1f")
        nc.sync.dma_start(out=w1_f, in_=io["w1f"][f])
        w1f_t[f] = w1_f

        hraw = hpool.tile([P, NBT, 4, BT], BF16, tag="hraw")
        hraw_t[f] = hraw
        for bt in range(NBT):
            bsl = slice(bt * BT, (bt + 1) * BT)
            for ec in range(4):
                psh = ps_mm.tile([P, BT], F32, tag="mm")
                for dc in range(4):
                    nc.tensor.matmul(psh, w1_t[:, dc, ec, :],
                                     xt_t[:, dc, bsl],
                                     start=(dc == 0), stop=(dc == 3))
                nc.scalar.activation(hraw[:, bt, ec, :], psh, AF.Gelu,
                                     bias=b1_t[:, ec:ec + 1])

    def phase_b(f, fi):
        """LN stats: musq/var on DVE (bf16 var is fine because downstream
        se and 1/se errors cancel in se*lam); one per-feature row Sqrt so
        phase C of this feature is not gated on the other feature's
        stats."""
        hraw = hraw_t[f]
        murow_f = rows.tile([1, NBT, BT], BF16, tag="murow")
        murow_t[f] = murow_f
        var_f = rows.tile([1, NBT, BT], BF16, tag="var")
        serow_f = rows.tile([1, NBT, BT], BF16, tag="serow")
        serow_t[f] = serow_f
        for bt in range(NBT):
            pmu = ps_st.tile([1, BT], F32, tag="pst")
            for ec in range(4):
                nc.tensor.matmul(pmu, ones_bf, hraw[:, bt, ec, :],
                                 start=(ec == 0), stop=(ec == 3))
            nc.vector.tensor_scalar_mul(murow_f[:, bt, :], pmu, 1.0 / D)
            musq = work.tile([1, BT], F32, tag="musq")
            nc.vector.tensor_mul(musq, murow_f[:, bt, :], murow_f[:, bt, :])

            pex = ps_st.tile([1, BT], F32, tag="pst")
            for ec in range(4):
                h2 = work.tile([P, BT], BF16, tag="h2")
                nc.vector.tensor_mul(h2, hraw[:, bt, ec, :],
                                     hraw[:, bt, ec, :])
                nc.tensor.matmul(pex, ones_bf, h2,
                                 start=(ec == 0), stop=(ec == 3))
            nc.vector.scalar_tensor_tensor(
                out=var_f[:, bt, :], in0=pex, scalar=1.0 / D,
                in1=musq, op0=OP.mult, op1=OP.subtract)
        # one sqrt per feature keeps the sqrt table set from sandwiching
        # into the Gelu/Exp stream at every batch tile
        nc.scalar.activation(serow_f, var_f, AF.Sqrt, bias=eps_t[0:1, 0:1])
        plam = ps_c8.tile([P, NS], F32, tag="lamc8")
        for bt in range(NBT):
            for bs in range(NBS):
                nc.tensor.matmul(plam[:, bt * NBS + bs:bt * NBS + bs + 1],
                                 serow_f[0:1, bt, bs * P:(bs + 1) * P],
                                 ones_bf[0:1, 0:1], start=True, stop=True)
        lamcol_f = rows.tile([P, NS], F32, tag="lamcol")
        nc.vector.reciprocal(lamcol_f, plam)
        lamcol_t[f] = lamcol_f

    def phase_c(f, fi):
        kind, j = feat_kind(f)
        b2row = b2row_of(f)
        hraw = hraw_t.pop(f)
        w1_f = w1f_t.pop(f)
        murow_f = murow_t.pop(f)
        serow_f = serow_t.pop(f)
        lamcol_f = lamcol_t.pop(f)
        if kind == "c":
            w2_f = wpool.tile([P, 2, V], BF16, tag="w2c")
            nc.sync.dma_start(out=w2_f, in_=io["w2c"][j])
        elif kind == "n":
            w2_f = wpool.tile([P, 2, D], BF16, tag="w2n")
            nc.sync.dma_start(out=w2_f, in_=io["w2n"][j])
        else:
            w2_f = wpool.tile([P, 2, 2], BF16, tag="w2b")
            nc.sync.dma_start(out=w2_f, in_=io["w2b"][j])

        for bt in range(NBT):
            bsl = slice(bt * BT, (bt + 1) * BT)
            hcT = hpool.tile([P, 2, BT], BF16, tag="hcT")
            for hc in range(2):
                psh1 = ps_h1.tile([P, BT], F32, tag="psh1")
                for dc in range(4):
                    nc.tensor.matmul(psh1, w1_f[:, dc, hc, :],
                                     hraw[:, bt, dc, :], start=(dc == 0),
                                     stop=False)
                nc.tensor.matmul(psh1, w1r_t[0:1, f, 0, hc * P:(hc + 1) * P],
                                 murow_f[:, bt, :], start=False, stop=False)
                nc.tensor.matmul(psh1, w1r_t[0:1, f, 1, hc * P:(hc + 1) * P],
                                 serow_f[0:1, bt, :], start=False,
                                 stop=True)
                nc.vector.tensor_scalar_max(hcT[:, hc, :], psh1, 0.0)

            if kind == "c":
                wct_t = xpool.tile([P, 2, BT], BF16, tag="wct")
                nc.sync.dma_start(out=wct_t, in_=io["wct"][j][:, :, bsl])
                prod = work.tile([P, 2, BT], BF16, tag="prod")
                nc.vector.tensor_mul(prod, hcT, wct_t)
                pqt = ps_st.tile([1, BT], F32, tag="pst")
                nc.tensor.matmul(pqt, ones_bf, prod[:, 0, :], start=True,
                                 stop=False)
                nc.tensor.matmul(pqt, ones_bf, prod[:, 1, :], start=False,
                                 stop=True)
                qtrow_bf = work.tile([1, BT], BF16, tag="qtrowbf")
                nc.vector.tensor_scalar_mul(qtrow_bf, pqt, 1.0)
                pc8 = ps_c8.tile([P, NBS], F32, tag="qtc8")
                for bs in range(NBS):
                    nc.tensor.matmul(pc8[:, bs:bs + 1],
                                     qtrow_bf[0:1, bs * P:(bs + 1) * P],
                                     ones_bf[0:1, 0:1], start=True, stop=True)

            for bs in range(NBS):
                sidx = bt * NBS + bs
                bpart = slice(bs * P, (bs + 1) * P)
                lam = lamcol_f[:, sidx:sidx + 1]
                serow_l = serow_f[0:1, bt, bpart]
                mk = mask_t[:, f, sidx:sidx + 1]

                if kind == "c":
                    scol = cols.tile([P, 2], F32, tag="scol")
                    for vi, vsl in enumerate((slice(0, 512), slice(512, V))):
                        psq = ps_mm.tile([P, BT], F32, tag="mm")
                        nv = vsl.stop - vsl.start
                        nc.tensor.matmul(psq[:, 0:nv], hcT[:, 0, bpart],
                                         w2_f[:, 0, vsl], start=True,
                                         stop=False)
                        nc.tensor.matmul(psq[:, 0:nv], hcT[:, 1, bpart],
                                         w2_f[:, 1, vsl], start=False,
                                         stop=False)
                        nc.tensor.matmul(psq[:, 0:nv], serow_l,
                                         b2row[0:1, vsl], start=False,
                                         stop=True)
                        u = work.tile([P, BT], BF16, tag="u")
                        nc.scalar.activation(u[:, 0:nv], psq[:, 0:nv], AF.Exp,
                                             scale=lam,
                                             accum_out=scol[:, vi:vi + 1])
                    nc.vector.tensor_add(ssum_all[:, f, sidx:sidx + 1],
                                         scol[:, 0:1], scol[:, 1:2])
                    nc.vector.scalar_tensor_tensor(
                        out=t1_all[:, f, sidx:sidx + 1],
                        in0=pc8[:, bs:bs + 1], scalar=lam,
                        in1=aux_t[:, f, sidx:sidx + 1],
                        op0=OP.mult, op1=OP.add)
                elif kind == "n":
                    psq = ps_mm.tile([P, BT], F32, tag="mm")
                    nc.tensor.matmul(psq, hcT[:, 0, bpart], w2_f[:, 0, :],
                                     start=True, stop=False)
                    nc.tensor.matmul(psq, hcT[:, 1, bpart], w2_f[:, 1, :],
                                     start=False, stop=False)
                    nc.tensor.matmul(psq, serow_l, b2row,
                                     start=False, stop=True)
                    tg = tgp.tile([P, D], BF16, tag="tg")
                    nc.sync.dma_start(
                        out=tg, in_=io["tgt"][j][bt * BT + bs * P:
                                                 bt * BT + (bs + 1) * P, :])
                    diff = work.tile([P, D], BF16, tag="diff")
                    nc.vector.scalar_tensor_tensor(
                        out=diff, in0=psq, scalar=lam, in1=tg,
                        op0=OP.mult, op1=OP.subtract)
                    sq = work.tile([P, D], BF16, tag="sq")
                    sse = cols.tile([P, 1], F32, tag="sse")
                    nc.vector.scalar_tensor_tensor(
                        out=sq, in0=diff, scalar=1.0, in1=diff,
                        op0=OP.bypass, op1=OP.mult, accum_out=sse)
                    nc.vector.scalar_tensor_tensor(
                        out=ceb_all[:, f, sidx:sidx + 1], in0=sse,
                        scalar=1.0 / D, in1=mk, op0=OP.mult, op1=OP.mult)
                else:
                    psq = ps_mm.tile([P, BT], F32, tag="mm")
                    nc.tensor.matmul(psq[:, 0:2], hcT[:, 0, bpart],
                                     w2_f[:, 0, :], start=True, stop=False)
                    nc.tensor.matmul(psq[:, 0:2], hcT[:, 1, bpart],
                                     w2_f[:, 1, :], start=False, stop=False)
                    nc.tensor.matmul(psq[:, 0:2], serow_l, b2row,
                                     start=False, stop=True)
                    u2 = cols.tile([P, 2], F32, tag="u2")
                    nc.scalar.activation(u2, psq[:, 0:2], AF.Exp, scale=lam,
                                         accum_out=ssum_all[:, f, sidx:sidx + 1])
                    l0 = cols.tile([P, 2], F32, tag="l0")
                    nc.vector.tensor_scalar(
                        out=l0, in0=psq[:, 0:2], scalar1=lam, scalar2=None,
                        op0=OP.mult)
                    dlt = cols.tile([P, 1], F32, tag="dlt")
                    nc.vector.tensor_sub(dlt, l0[:, 1:2], l0[:, 0:1])
                    nc.vector.scalar_tensor_tensor(
                        out=t1_all[:, f, sidx:sidx + 1], in0=dlt,
                        scalar=aux_t[:, f, sidx:sidx + 1], in1=l0[:, 0:1],
                        op0=OP.mult, op1=OP.add)

    # ---- grouped phase driver: 2 same-kind features per group so each
    # activation-table set loads once per phase, and the scheduler can
    # interleave freely within a group without table thrash ----
    def group_tail(group):
        g0 = group[0]
        kind = feat_kind(g0)[0]
        if kind in ("c", "b"):
            lns_g = fin.tile([P, 2, NS], F32, tag="lns")
            nc.scalar.activation(lns_g, ssum_all[:, g0:g0 + 2, :], AF.Ln)
            for fi, f in enumerate(group):
                tmp = fin.tile([P, NS], F32, tag="tmp")
                nc.vector.tensor_sub(tmp, lns_g[:, fi, :], t1_all[:, f, :])
                nc.vector.tensor_mul(ceb_all[:, f, :], tmp, mask_t[:, f, :])
        for f in group:
            nc.vector.reduce_sum(ceacc[:, f:f + 1], ceb_all[:, f, :],
                                 axis=mybir.AxisListType.X)

    for group in ([0, 1], [2, 3], [4, 5], [6, 7]):
        for f in group:
            phase_a(f)
        for fi, f in enumerate(group):
            phase_b(f, fi)
        for fi, f in enumerate(group):
            phase_c(f, fi)
        group_tail(group)

    # final partition reduction happens host-side in combine()
    nc.sync.dma_start(out=out, in_=ceacc)


_NC_CACHE = None


def _get_nc():
    global _NC_CACHE
    if _NC_CACHE is None:
        _NC_CACHE = _build()
    return _NC_CACHE


def _bf(a):
    return np.ascontiguousarray(a.astype(bfloat16))


def _prep_core(i, seq, targets, mask_f, cat_t, bool_t, w):
    cg = list(range(4 * i, 4 * i + 4))
    ng = list(range(2 * i, 2 * i + 2))
    bg = list(range(2 * i, 2 * i + 2))
    feats = cg + [NC + g for g in ng] + [NC + NN + g for g in bg]

    xt = np.ascontiguousarray(seq[:, feats, :].transpose(1, 2, 0))

    # mask/aux in [P, NF, NS] layout
    mask_all = np.zeros((P, NF, NS), np.float32)
    aux_all = np.zeros((P, NF, NS), np.float32)
    for k, g in enumerate(feats):
        mask_all[:, k, :] = mask_f[:, g].reshape(NS, P).T
    for k, g in enumerate(cg):
        aux_all[:, k, :] = w["bc2"][g][cat_t[:, g]].reshape(NS, P).T
    for k, g in enumerate(bg):
        aux_all[:, FC + FN + k, :] = bool_t[:, g].astype(
            np.float32).reshape(NS, P).T

    b2all = np.concatenate([
        w["bc2"][cg].ravel(), w["bn2"][ng].ravel(), w["bb2"][bg].ravel()])

    wct = np.stack([
        w["Wc2"][g][:, cat_t[:, g]].reshape(2, P, B).transpose(1, 0, 2)
        for g in cg])

    m = {
        "xt": _bf(xt),
        "w1t": _bf(w["W1"].reshape(4, P, 4, P).transpose(1, 0, 2, 3)),
        "b1c": np.ascontiguousarray(w["b1"].reshape(4, P).T),
        "w1f": _bf(np.stack([
            w["w1p"][k].reshape(4, P, 2, P).transpose(1, 0, 2, 3)
            for k in range(NF)])),
        "w1r": _bf(w["w1rows"][None]),
        "w2c": _bf(np.stack([
            w["Wc2"][g].reshape(2, P, V).transpose(1, 0, 2) for g in cg])),
        "wct": _bf(wct),
        "w2n": _bf(np.stack([
            w["Wn2"][g].reshape(2, P, D).transpose(1, 0, 2) for g in ng])),
        "w2b": _bf(np.stack([
            w["Wb2"][g].reshape(2, P, 2).transpose(1, 0, 2) for g in bg])),
        "b2all": _bf(b2all[None]),
        "tgt": _bf(np.ascontiguousarray(
            targets[:, [NC + g for g in ng], :].transpose(1, 0, 2))),
        "mask_all": mask_all,
        "aux_all": aux_all,
    }
    return m


def prepare_in_maps(inputs):
    seq = np.asarray(inputs["sequence_embeddings"], np.float32)
    targets = np.asarray(inputs["targets"], np.float32)
    mask_f = np.asarray(inputs["target_mask"]).astype(np.float32)
    cat_t = np.asarray(inputs["cat_targets"]).astype(np.int64)
    bool_t = np.asarray(inputs["bool_targets"]).astype(np.int64)

    ln_g = np.asarray(inputs["ln_g"], np.float64)
    ln_b = np.asarray(inputs["ln_b"], np.float64)

    def fold(w1, b1):
        w1 = np.asarray(w1, np.float64)
        b1 = np.asarray(b1, np.float64)
        wp = ln_g[None, :, None] * w1
        bp = b1 + np.einsum("d,fdh->fh", ln_b, w1)
        rows = np.stack([-wp.sum(1), bp], axis=1)
        return wp.astype(np.float32), rows.astype(np.float32)

    w = {
        "W1": np.asarray(inputs["W1"], np.float32),
        "b1": np.asarray(inputs["b1"], np.float32),
        "Wc2": np.asarray(inputs["Wc2"], np.float32),
        "bc2": np.asarray(inputs["bc2"], np.float32),
        "Wn2": np.asarray(inputs["Wn2"], np.float32),
        "bn2": np.asarray(inputs["bn2"], np.float32),
        "Wb2": np.asarray(inputs["Wb2"], np.float32),
        "bb2": np.asarray(inputs["bb2"], np.float32),
    }
    wc1p, wc1r = fold(inputs["Wc1"], inputs["bc1"])
    wn1p, wn1r = fold(inputs["Wn1"], inputs["bn1"])
    wb1p, wb1r = fold(inputs["Wb1"], inputs["bb1"])

    maps = []
    for i in range(N_CORES):
        cg = list(range(4 * i, 4 * i + 4))
        ng = list(range(2 * i, 2 * i + 2))
        bg = list(range(2 * i, 2 * i + 2))
        wi = dict(w)
        wi["w1p"] = np.concatenate(
            [wc1p[cg], wn1p[ng], wb1p[bg]])
        wi["w1rows"] = np.concatenate(
            [wc1r[cg], wn1r[ng], wb1r[bg]])
        maps.append(_prep_core(i, seq, targets, mask_f, cat_t, bool_t, wi))
    return maps


def combine(per_core_outs):
    total = 0.0
    for r in per_core_outs:
        r = np.asarray(r, np.float64).reshape(P, 2 * NF).sum(0)
        s, c = r[:NF], r[NF:]
        total += np.where(c > 0, s / np.maximum(c, 1.0), 0.0).sum()
    return np.float32(total)


def kernel(**inputs):
    global LAST_RESULTS
    in_maps = prepare_in_maps(inputs)
    nc = _get_nc()
    res = run_bass_kernel_spmd(nc, in_maps, core_ids=list(range(N_CORES)))
    LAST_RESULTS = res
    return combine([res.results[i]["loss_out"] for i in range(N_CORES)])
